# revision 1
# baseline (speedup 1.0000x reference)
"""Trainium2 Bass kernel for CustomGNN (fc+relu -> SAGEConv+relu -> SAGEConv -> head).

Strategy (8 NeuronCores, SPMD — one program, per-core data):
  - dst-nodes sharded in 8 contiguous chunks of 12500; edges partitioned by dst owner.
  - fc stage replicated: every core computes the full h table (reads full x).
  - mean-aggregation: edges sorted by (dst-window, src-chunk, dst); messages fetched
    with dma_gather (int16 indices relative to 4 src-chunk bases of 25000 rows);
    segment-sum via PE matmuls against one-hot*inv_deg matrices built on DVE
    (is_equal vs iota over 32-wide sliding seg windows); PSUM window [64, 512 dst].
  - two NEFFs: phase A emits h1 chunks, host concatenates (data movement only),
    phase B gathers from full h1 and emits the head output.

All index-derived structure (column quotas R, seg-window starts F) is computed on the
host from edge_index and baked into the program; quotas are max-merged across the 8
cores so the SPMD program is identical.
"""
import math
import time
import numpy as np

from concourse import bacc, mybir
from concourse.tile import TileContext
from concourse.bass_utils import run_bass_kernel_spmd

timings = {}

NSWQ = 2

F32 = mybir.dt.float32
I16 = mybir.dt.int16


class Cfg:
    def __init__(self, n_nodes=100000, in_dim=128, hid=64, ncores=8,
                 win=512, nchunk=4, segw=32, colb=8, xb=4):
        assert n_nodes % ncores == 0
        self.N = n_nodes
        self.D = in_dim
        self.H = hid
        self.NC = ncores
        self.NPC = n_nodes // ncores
        self.WIN = win
        self.NWIN = math.ceil(self.NPC / win)
        self.NCHUNK = nchunk
        self.CHUNK = math.ceil(n_nodes / nchunk)
        assert self.CHUNK <= 32767
        self.SEGW = segw
        self.COLB = colb
        self.XB = xb


def _schedule_F(R, span, segw):
    if R <= 1:
        return [0] * max(R, 1)
    top = max(span - segw, 0)
    return [min((j * top) // (R - 1), top) for j in range(R)]


def _pack_group(segs, R, F, segw):
    """Greedy-pack sorted segs into R cols of <=128 edges, col j window [F[j], F[j]+segw).
    Returns list of (start, count) per col, or None if infeasible."""
    n = len(segs)
    ptr = 0
    out = []
    for j in range(R):
        if ptr < n and segs[ptr] < F[j]:
            return None  # lagging edge can't be placed
        hi = np.searchsorted(segs, F[j] + segw, side="left")
        take = min(128, hi - ptr)
        out.append((ptr, take))
        ptr += take
    if ptr != n:
        return None
    return out


def plan(cfg, src, dst):
    """Host planning. Returns structure (uniform across cores) + per-core data arrays."""
    N, NC, NPC, WIN, NWIN = cfg.N, cfg.NC, cfg.NPC, cfg.WIN, cfg.NWIN
    NCHUNK, CHUNK, SEGW = cfg.NCHUNK, cfg.CHUNK, cfg.SEGW

    deg = np.bincount(dst, minlength=N)
    inv = (1.0 / np.maximum(deg, 1)).astype(np.float32)

    core = dst // NPC
    local = dst - core * NPC
    win = local // WIN
    seg = (local - win * WIN).astype(np.int32)
    chunk = src // CHUNK
    srcrel = (src - chunk * CHUNK).astype(np.int32)

    gid = ((core * NWIN + win) * NCHUNK + chunk).astype(np.int64)
    order = np.lexsort((seg, gid))
    gids = gid[order]
    segs_all = seg[order]
    srcrel_all = srcrel[order]
    invdst_all = inv[dst[order]].astype(np.float32)

    ngroups = NC * NWIN * NCHUNK
    counts = np.bincount(gids, minlength=ngroups)
    starts = np.zeros(ngroups + 1, dtype=np.int64)
    np.cumsum(counts, out=starts[1:])

    def span_of(w):
        return min(WIN, NPC - w * WIN)

    # uniform R[w][c] across cores, bumped until packing feasible for every core
    R = [[0] * NCHUNK for _ in range(NWIN)]
    F = [[None] * NCHUNK for _ in range(NWIN)]
    packs = {}  # (k, w, c) -> cols list
    for w in range(NWIN):
        span = span_of(w)
        for c in range(NCHUNK):
            r = 1
            for k in range(NC):
                g = (k * NWIN + w) * NCHUNK + c
                r = max(r, math.ceil(max(counts[g], 1) / 128))
            while True:
                f = _schedule_F(r, span, SEGW)
                ok = True
                for k in range(NC):
                    g = (k * NWIN + w) * NCHUNK + c
                    segs = segs_all[starts[g]:starts[g + 1]]
                    p = _pack_group(segs, r, f, SEGW)
                    if p is None:
                        ok = False
                        break
                    packs[(k, w, c)] = p
                if ok:
                    break
                r += 1
            R[w][c] = r
            F[w][c] = f

    col_off = [[0] * NCHUNK for _ in range(NWIN)]  # column offset of group (w,c)
    tot = 0
    for w in range(NWIN):
        for c in range(NCHUNK):
            col_off[w][c] = tot
            tot += R[w][c]
    totcols = tot

    # per-core data arrays
    idx_wrapped = []   # [128, totcols*8] int16
    segv = []          # [128, 2, totcols] f32
    for k in range(NC):
        idxk = np.zeros((totcols, 128), dtype=np.int16)   # [col, slot]
        segk = np.zeros((totcols, 128), dtype=np.float32)
        vk = np.zeros((totcols, 128), dtype=np.float32)
        for w in range(NWIN):
            for c in range(NCHUNK):
                g = (k * NWIN + w) * NCHUNK + c
                s0 = starts[g]
                f = F[w][c]
                for j, (ptr, take) in enumerate(packs[(k, w, c)]):
                    if take <= 0:
                        continue
                    col = col_off[w][c] + j
                    sl = slice(s0 + ptr, s0 + ptr + take)
                    idxk[col, :take] = srcrel_all[sl].astype(np.int16)
                    segk[col, :take] = (segs_all[sl] - f[j]).astype(np.float32)
                    vk[col, :take] = invdst_all[sl]
        # wrap idx: per column, linear slot s -> sbuf[s%16, scol*8 + s//16]
        iw = idxk.reshape(totcols, 8, 16).transpose(2, 0, 1).reshape(16, totcols * 8)
        idx_wrapped.append(np.tile(iw, (8, 1)))
        segv.append(np.stack([segk.T, vk.T], axis=1))  # [128, 2, totcols]

    return dict(R=R, F=F, col_off=col_off, totcols=totcols,
                idx=idx_wrapped, segv=segv, span_of=span_of)


def pack_weights(cfg, fc_W, fc_b, Wl1, bl1, Wr1, Wl2, bl2, Wr2, head_W, head_b):
    """One [128, WCOLS] f32 tensor. Layout columns:
       0:D        identity [DxD at top-left of 128x128 block]
       D:D+SEGW   iota (0..SEGW-1 replicated per partition)
       then H-wide blocks: fcWT[D,H], Wl1T, Wr1T, Wl2T, Wr2T (H rows),
       headWT col, bl1 col, fcb col, fcb_rep [128? H free]"""
    D, H, SEGW = cfg.D, cfg.H, cfg.SEGW
    cols = {}
    c = 0
    wt = np.zeros((128, D + SEGW + H * 5 + 4 + H), dtype=np.float32)
    wt[:D, 0:D] = np.eye(D, dtype=np.float32)
    cols["ident"] = 0
    c = D
    wt[:, c:c + SEGW] = np.tile(np.arange(SEGW, dtype=np.float32), (128, 1))
    cols["iota"] = c
    c += SEGW
    for name, m in [("fcWT", fc_W.T), ("Wl1T", Wl1.T), ("Wr1T", Wr1.T),
                    ("Wl2T", Wl2.T), ("Wr2T", Wr2.T)]:
        r, cc = m.shape
        wt[:r, c:c + cc] = m
        cols[name] = c
        c += cc
    wt[:H, c] = head_W[0]
    cols["headWT"] = c
    c += 1
    wt[:H, c] = bl1
    cols["bl1"] = c
    c += 1
    wt[:H, c] = fc_b
    cols["fcb"] = c
    c += 1
    wt[:, c:c + H] = np.tile(fc_b, (128, 1))
    cols["fcb_rep"] = c
    c += H
    head_b_eff = float(head_b[0] + head_W[0] @ bl2)
    wt[0, c] = head_b_eff
    cols["hbe"] = c
    c += 1
    return wt, cols, head_b_eff


def build_phase(cfg, pl, wcols, wcols_n, phase, head_b_eff=0.0, reps=1):
    """phase 'A': inputs x, x_my, idx, segv, wt -> output h1_my [NPC, H]
       phase 'B': inputs h1_full, h1T_my, idx, segv, wt -> output out [1, NPC]"""
    N, D, H, NPC, WIN, NWIN = cfg.N, cfg.D, cfg.H, cfg.NPC, cfg.WIN, cfg.NWIN
    NCHUNK, CHUNK, SEGW, COLB, XB = cfg.NCHUNK, cfg.CHUNK, cfg.SEGW, cfg.COLB, cfg.XB
    R, F, col_off, totcols = pl["R"], pl["F"], pl["col_off"], pl["totcols"]
    span_of = pl["span_of"]

    nc = bacc.Bacc(None, target_bir_lowering=False, num_swdge_queues=NSWQ)
    wt_d = nc.dram_tensor("wt", [128, wcols_n], F32, kind="ExternalInput")
    idx_d = nc.dram_tensor("idx", [128, totcols * 8], I16, kind="ExternalInput")
    segv_d = nc.dram_tensor("segv", [128, 2, totcols], F32, kind="ExternalInput")

    if phase == "A":
        x_d = nc.dram_tensor("x", [N, D], F32, kind="ExternalInput")
        xmy_d = nc.dram_tensor("x_my", [NPC, D], F32, kind="ExternalInput")
        h1_d = nc.dram_tensor("h1_my", [NPC, H], F32, kind="ExternalOutput")
        htab_d = nc.dram_tensor("htab", [N, H], F32, kind="Internal")
    else:
        htab_d = nc.dram_tensor("h1_full", [N, H], F32, kind="ExternalInput")
        hTmy_d = nc.dram_tensor("h1T_my", [H, NPC], F32, kind="ExternalInput")
        out_d = nc.dram_tensor("out", [1, NPC], F32, kind="ExternalOutput")

    with TileContext(nc) as tc:
        with (
            tc.tile_pool(name="const", bufs=1) as constp,
            tc.tile_pool(name="resid", bufs=1) as resid,
            tc.tile_pool(name="io", bufs=3) as iop,
            tc.tile_pool(name="gat", bufs=2) as gatp,
            tc.tile_pool(name="bb", bufs=3) as bbp,
            tc.tile_pool(name="ps", bufs=4, space="PSUM") as psp,
            tc.tile_pool(name="ps2", bufs=2, space="PSUM") as psp2,
        ):
            wt = constp.tile([128, wcols_n], F32)
            nc.sync.dma_start(wt[:], wt_d[:, :])
            ident = wt[:, wcols["ident"]:wcols["ident"] + D]
            iota = wt[:, wcols["iota"]:wcols["iota"] + SEGW]
            fcWT = wt[:D, wcols["fcWT"]:wcols["fcWT"] + H]
            Wl1T = wt[:H, wcols["Wl1T"]:wcols["Wl1T"] + H]
            Wr1T = wt[:H, wcols["Wr1T"]:wcols["Wr1T"] + H]
            Wl2T = wt[:H, wcols["Wl2T"]:wcols["Wl2T"] + H]
            Wr2T = wt[:H, wcols["Wr2T"]:wcols["Wr2T"] + H]
            headWT = wt[:H, wcols["headWT"]:wcols["headWT"] + 1]
            bl1 = wt[:H, wcols["bl1"]:wcols["bl1"] + 1]
            fcb = wt[:H, wcols["fcb"]:wcols["fcb"] + 1]
            fcb_rep = wt[:, wcols["fcb_rep"]:wcols["fcb_rep"] + H]

            hT = resid.tile([H, NPC], F32)  # transposed features of my nodes

            for _rep in range(reps):
                if phase == "A":
                    # ---- fc stage: full h table (replicated across cores) ----
                    nt = math.ceil(N / 128)
                    for i0 in range(0, nt, XB):
                        nb = min(XB, nt - i0)
                        rows0 = i0 * 128
                        rowsn = min(N - rows0, nb * 128)
                        xb_t = iop.tile([128, XB, D], F32, tag="xb")
                        nfull = rowsn // 128
                        if nfull:
                            nc.sync.dma_start(
                                xb_t[:, :nfull, :],
                                x_d[rows0:rows0 + nfull * 128, :].rearrange(
                                    "(t p) d -> p t d", p=128))
                        rem = rowsn - nfull * 128
                        if rem:
                            nc.sync.dma_start(xb_t[:rem, nfull, :],
                                              x_d[rows0 + nfull * 128:rows0 + rowsn, :])
                        hb_t = iop.tile([128, XB, H], F32, tag="hb")
                        for t in range(nb):
                            rows = min(128, N - rows0 - t * 128)
                            xT_ps = psp.tile([128, D], F32, space="PSUM", tag="s")
                            nc.tensor.transpose(xT_ps[:, :rows], xb_t[:rows, t, :],
                                                ident[:rows, :rows])
                            xT = iop.tile([128, D], F32, tag="xT_sb")
                            nc.vector.tensor_copy(xT[:], xT_ps[:])
                            h_ps = psp.tile([128, H], F32, space="PSUM", tag="s")
                            nc.tensor.matmul(out=h_ps[:rows, :], lhsT=xT[:, :rows],
                                             rhs=fcWT, start=True, stop=True)
                            nc.vector.tensor_tensor(out=hb_t[:rows, t, :], in0=h_ps[:rows, :],
                                                    in1=fcb_rep[:rows, :],
                                                    op=mybir.AluOpType.add)
                            nc.scalar.activation(hb_t[:rows, t, :], hb_t[:rows, t, :],
                                                 mybir.ActivationFunctionType.Relu)
                        if nfull:
                            nc.sync.dma_start(
                                htab_d[rows0:rows0 + nfull * 128, :].rearrange(
                                    "(t p) d -> p t d", p=128),
                                hb_t[:, :nfull, :])
                        if rem:
                            nc.sync.dma_start(htab_d[rows0 + nfull * 128:rows0 + rowsn, :],
                                              hb_t[:rem, nfull, :])

                    # ---- hT for my nodes (recompute from x_my) ----
                    for i in range(math.ceil(NPC / 128)):
                        rows = min(128, NPC - i * 128)
                        xm_t = iop.tile([128, D], F32, tag="xm")
                        nc.sync.dma_start(xm_t[:rows, :], xmy_d[i * 128:i * 128 + rows, :])
                        xT_ps = psp.tile([128, D], F32, space="PSUM", tag="s")
                        nc.tensor.transpose(xT_ps[:, :rows], xm_t[:rows, :], ident[:rows, :rows])
                        xT = iop.tile([128, D], F32, tag="xT_sb")
                        nc.vector.tensor_copy(xT[:], xT_ps[:])
                        hT_ps = psp.tile([H, 128], F32, space="PSUM", tag="s")
                        nc.tensor.matmul(out=hT_ps[:, :rows], lhsT=fcWT, rhs=xT[:, :rows],
                                         start=True, stop=True)
                        nc.scalar.activation(hT[:, i * 128:i * 128 + rows], hT_ps[:, :rows],
                                             mybir.ActivationFunctionType.Relu, bias=fcb)
                else:
                    nc.sync.dma_start(hT[:], hTmy_d[:, :])
                    out_sb = resid.tile([1, NPC], F32)

                WlT = Wl1T if phase == "A" else Wl2T
                WrT = Wr1T if phase == "A" else Wr2T

                # ---- window loop: gather + segment-sum + SAGE linear ----
                for w in range(NWIN):
                    span = span_of(w)
                    cw0 = col_off[w][0]
                    cwn = col_off[w][NCHUNK - 1] + R[w][NCHUNK - 1] - cw0

                    idx_w = iop.tile([128, cwn * 8], I16, tag="idxw")
                    nc.sync.dma_start(idx_w[:], idx_d[:, cw0 * 8:(cw0 + cwn) * 8])
                    segv_w = iop.tile([128, 2, cwn], F32, tag="segvw")
                    nc.sync.dma_start(segv_w[:], segv_d[:, :, cw0:cw0 + cwn])

                    agg_ps = psp2.tile([H, WIN], F32, space="PSUM", tag="agg")
                    nc.vector.memset(agg_ps[:, :], 0.0)

                    mm = []
                    for c in range(NCHUNK):
                        r = R[w][c]
                        o = col_off[w][c] - cw0  # local col offset within window
                        wg = gatp.tile([128, r, H], F32, tag=f"wg{c % 2}")
                        nc.gpsimd.dma_gather(
                            out_ap=wg[:, :, :],
                            in_ap=htab_d[c * CHUNK:min((c + 1) * CHUNK, N), :],
                            idxs_ap=idx_w[:, o * 8:(o + r) * 8],
                            num_idxs=r * 128, num_idxs_reg=r * 128, elem_size=H,
                            single_packet=False)
                        for b0 in range(0, r, COLB):
                            nb = min(COLB, r - b0)
                            B = bbp.tile([128, COLB, SEGW], F32, tag="B")
                            seg_b = segv_w[:, 0, o + b0:o + b0 + nb].unsqueeze(2) \
                                .to_broadcast([128, nb, SEGW])
                            v_b = segv_w[:, 1, o + b0:o + b0 + nb].unsqueeze(2) \
                                .to_broadcast([128, nb, SEGW])
                            iota_b = iota.unsqueeze(1).to_broadcast([128, nb, SEGW])
                            nc.vector.tensor_tensor(out=B[:, :nb, :], in0=seg_b, in1=iota_b,
                                                    op=mybir.AluOpType.is_equal)
                            nc.vector.tensor_tensor(out=B[:, :nb, :], in0=B[:, :nb, :],
                                                    in1=v_b, op=mybir.AluOpType.mult)
                            for t in range(nb):
                                j = b0 + t
                                f = F[w][c][j]
                                sw = min(SEGW, span - f)
                                mm.append((wg, j, B, t, f, sw))
                    for q, (wg, j, B, t, f, sw) in enumerate(mm):
                        nc.tensor.matmul(out=agg_ps[:, f:f + sw], lhsT=wg[:, j, :],
                                         rhs=B[:, t, :sw], start=False,
                                         stop=(q == len(mm) - 1), skip_group_check=True)

                    aggT = iop.tile([H, WIN], F32, tag="aggT")
                    nc.scalar.copy(aggT[:, :span], agg_ps[:, :span])

                    z_ps = psp2.tile([H, WIN], F32, space="PSUM", tag="z")
                    nc.tensor.matmul(out=z_ps[:, :span], lhsT=WlT, rhs=aggT[:, :span],
                                     start=True, stop=False)
                    nc.tensor.matmul(out=z_ps[:, :span], lhsT=WrT,
                                     rhs=hT[:, w * WIN:w * WIN + span],
                                     start=False, stop=True)

                    if phase == "A":
                        h1T_sb = iop.tile([H, WIN], F32, tag="h1T")
                        nc.scalar.activation(h1T_sb[:, :span], z_ps[:, :span],
                                             mybir.ActivationFunctionType.Relu, bias=bl1)
                        for b in range(math.ceil(span / 128)):
                            rows = min(128, span - b * 128)
                            tr_ps = psp.tile([128, H], F32, space="PSUM", tag="s")
                            nc.tensor.transpose(tr_ps[:rows, :], h1T_sb[:, b * 128:b * 128 + rows],
                                                ident[:H, :H])
                            h1_t = iop.tile([128, H], F32, tag="h1t")
                            nc.vector.tensor_copy(h1_t[:rows, :], tr_ps[:rows, :])
                            nc.sync.dma_start(
                                h1_d[w * WIN + b * 128:w * WIN + b * 128 + rows, :],
                                h1_t[:rows, :])
                    else:
                        h2T_sb = iop.tile([H, WIN], F32, tag="h2T")
                        nc.scalar.copy(h2T_sb[:, :span], z_ps[:, :span])
                        o_ps = psp.tile([1, WIN], F32, space="PSUM", tag="s")
                        nc.tensor.matmul(out=o_ps[:, :span], lhsT=headWT,
                                         rhs=h2T_sb[:, :span], start=True, stop=True)
                        nc.scalar.activation(out_sb[:, w * WIN:w * WIN + span],
                                             o_ps[:, :span],
                                             mybir.ActivationFunctionType.Identity,
                                             bias=wt[0:1, wcols["hbe"]:wcols["hbe"] + 1])

            if phase == "B":
                nc.sync.dma_start(out_d[:, :], out_sb[:, :])

    nc.compile()
    return nc


def kernel(x, edge_index, batch, fc_W, fc_b, Wl1, bl1, Wr1, Wl2, bl2, Wr2,
           head_W, head_b, cfg=None):
    cfg = cfg or Cfg(n_nodes=x.shape[0], in_dim=x.shape[1], hid=fc_W.shape[0])
    x = np.asarray(x, dtype=np.float32)
    src = np.asarray(edge_index[0], dtype=np.int64)
    dst = np.asarray(edge_index[1], dtype=np.int64)
    fc_W, fc_b, Wl1, bl1, Wr1, Wl2, bl2, Wr2, head_W, head_b = [
        np.asarray(a, dtype=np.float32)
        for a in (fc_W, fc_b, Wl1, bl1, Wr1, Wl2, bl2, Wr2, head_W, head_b)]

    t0 = time.time()
    pl = plan(cfg, src, dst)
    timings["plan_s"] = time.time() - t0
    wt, wcols, head_b_eff = pack_weights(cfg, fc_W, fc_b, Wl1, bl1, Wr1,
                                         Wl2, bl2, Wr2, head_W, head_b)

    t0 = time.time()
    ncA = build_phase(cfg, pl, wcols, wt.shape[1], "A")
    timings["buildA_s"] = time.time() - t0
    in_maps_A = []
    for k in range(cfg.NC):
        in_maps_A.append({
            "wt": wt, "idx": pl["idx"][k], "segv": pl["segv"][k],
            "x": x, "x_my": x[k * cfg.NPC:(k + 1) * cfg.NPC],
        })
    t0 = time.time()
    resA = run_bass_kernel_spmd(ncA, in_maps_A, core_ids=list(range(cfg.NC)))
    timings["runA_s"] = time.time() - t0
    timings["hwA_ns"] = resA.exec_time_ns
    h1 = np.concatenate([r["h1_my"] for r in resA.results], axis=0)

    t0 = time.time()
    ncB = build_phase(cfg, pl, wcols, wt.shape[1], "B", head_b_eff=head_b_eff)
    timings["buildB_s"] = time.time() - t0
    in_maps_B = []
    for k in range(cfg.NC):
        in_maps_B.append({
            "wt": wt, "idx": pl["idx"][k], "segv": pl["segv"][k],
            "h1_full": h1,
            "h1T_my": np.ascontiguousarray(h1[k * cfg.NPC:(k + 1) * cfg.NPC].T),
        })
    t0 = time.time()
    resB = run_bass_kernel_spmd(ncB, in_maps_B, core_ids=list(range(cfg.NC)))
    timings["runB_s"] = time.time() - t0
    timings["hwB_ns"] = resB.exec_time_ns
    out = np.concatenate([r["out"][0] for r in resB.results], axis=0)
    return out.reshape(cfg.N, 1).astype(np.float32)



# revision 3
# speedup vs baseline: 2.1491x; 2.1491x over previous
"""Trainium2 Bass kernel for CustomGNN (fc+relu -> SAGEConv+relu -> SAGEConv -> head).

Strategy (8 NeuronCores, SPMD — one program, per-core data):
  - dst-nodes sharded in 8 contiguous chunks of 12500; edges partitioned by dst owner.
  - fc stage replicated: every core computes the full h table (reads full x).
  - mean-aggregation: edges sorted by (dst-window, src-chunk, dst); messages fetched
    with dma_gather (int16 indices relative to 4 src-chunk bases of 25000 rows);
    segment-sum via PE matmuls against one-hot*inv_deg matrices built on DVE
    (is_equal vs iota over 32-wide sliding seg windows); PSUM window [64, 512 dst].
  - two NEFFs: phase A emits h1 chunks, host concatenates (data movement only),
    phase B gathers from full h1 and emits the head output.

All index-derived structure (column quotas R, seg-window starts F) is computed on the
host from edge_index and baked into the program; quotas are max-merged across the 8
cores so the SPMD program is identical.
"""
import math
import time
import numpy as np

from concourse import bacc, mybir
from concourse.tile import TileContext
from concourse.bass_utils import run_bass_kernel_spmd

timings = {}

NSWQ = 4

F32 = mybir.dt.float32
I16 = mybir.dt.int16


class Cfg:
    def __init__(self, n_nodes=100000, in_dim=128, hid=64, ncores=8,
                 win=512, nchunk=4, segw=32, colb=8, xb=4):
        assert n_nodes % ncores == 0
        self.N = n_nodes
        self.D = in_dim
        self.H = hid
        self.NC = ncores
        self.NPC = n_nodes // ncores
        self.WIN = win
        self.NWIN = math.ceil(self.NPC / win)
        self.NCHUNK = nchunk
        self.CHUNK = math.ceil(n_nodes / nchunk)
        assert self.CHUNK <= 32767
        self.SEGW = segw
        self.COLB = colb
        self.XB = xb


def _schedule_F(R, span, segw):
    if R <= 1:
        return [0] * max(R, 1)
    top = max(span - segw, 0)
    return [min((j * top) // (R - 1), top) for j in range(R)]


def _pack_group(segs, R, F, segw):
    """Greedy-pack sorted segs into R cols of <=128 edges, col j window [F[j], F[j]+segw).
    Returns list of (start, count) per col, or None if infeasible."""
    n = len(segs)
    ptr = 0
    out = []
    for j in range(R):
        if ptr < n and segs[ptr] < F[j]:
            return None  # lagging edge can't be placed
        hi = np.searchsorted(segs, F[j] + segw, side="left")
        take = min(128, hi - ptr)
        out.append((ptr, take))
        ptr += take
    if ptr != n:
        return None
    return out


def plan(cfg, src, dst):
    """Host planning. Returns structure (uniform across cores) + per-core data arrays."""
    N, NC, NPC, WIN, NWIN = cfg.N, cfg.NC, cfg.NPC, cfg.WIN, cfg.NWIN
    NCHUNK, CHUNK, SEGW = cfg.NCHUNK, cfg.CHUNK, cfg.SEGW

    deg = np.bincount(dst, minlength=N)
    inv = (1.0 / np.maximum(deg, 1)).astype(np.float32)

    core = dst // NPC
    local = dst - core * NPC
    win = local // WIN
    seg = (local - win * WIN).astype(np.int32)
    chunk = src // CHUNK
    srcrel = (src - chunk * CHUNK).astype(np.int32)

    gid = ((core * NWIN + win) * NCHUNK + chunk).astype(np.int64)
    order = np.lexsort((seg, gid))
    gids = gid[order]
    segs_all = seg[order]
    srcrel_all = srcrel[order]
    invdst_all = inv[dst[order]].astype(np.float32)

    ngroups = NC * NWIN * NCHUNK
    counts = np.bincount(gids, minlength=ngroups)
    starts = np.zeros(ngroups + 1, dtype=np.int64)
    np.cumsum(counts, out=starts[1:])

    def span_of(w):
        return min(WIN, NPC - w * WIN)

    # uniform R[w][c] across cores, bumped until packing feasible for every core
    R = [[0] * NCHUNK for _ in range(NWIN)]
    F = [[None] * NCHUNK for _ in range(NWIN)]
    packs = {}  # (k, w, c) -> cols list
    for w in range(NWIN):
        span = span_of(w)
        for c in range(NCHUNK):
            r = 1
            for k in range(NC):
                g = (k * NWIN + w) * NCHUNK + c
                r = max(r, math.ceil(max(counts[g], 1) / 128))
            while True:
                f = _schedule_F(r, span, SEGW)
                ok = True
                for k in range(NC):
                    g = (k * NWIN + w) * NCHUNK + c
                    segs = segs_all[starts[g]:starts[g + 1]]
                    p = _pack_group(segs, r, f, SEGW)
                    if p is None:
                        ok = False
                        break
                    packs[(k, w, c)] = p
                if ok:
                    break
                r += 1
            R[w][c] = r
            F[w][c] = f

    col_off = [[0] * NCHUNK for _ in range(NWIN)]  # column offset of group (w,c)
    tot = 0
    for w in range(NWIN):
        for c in range(NCHUNK):
            col_off[w][c] = tot
            tot += R[w][c]
    totcols = tot

    # per-core data arrays
    idx_wrapped = []   # [128, totcols*8] int16
    segv = []          # [128, 2, totcols] f32
    for k in range(NC):
        idxk = np.zeros((totcols, 128), dtype=np.int16)   # [col, slot]
        segk = np.zeros((totcols, 128), dtype=np.float32)
        vk = np.zeros((totcols, 128), dtype=np.float32)
        for w in range(NWIN):
            for c in range(NCHUNK):
                g = (k * NWIN + w) * NCHUNK + c
                s0 = starts[g]
                f = F[w][c]
                for j, (ptr, take) in enumerate(packs[(k, w, c)]):
                    if take <= 0:
                        continue
                    col = col_off[w][c] + j
                    sl = slice(s0 + ptr, s0 + ptr + take)
                    idxk[col, :take] = srcrel_all[sl].astype(np.int16)
                    segk[col, :take] = (segs_all[sl] - f[j]).astype(np.float32)
                    vk[col, :take] = invdst_all[sl]
        # wrap idx: per column, linear slot s -> sbuf[s%16, scol*8 + s//16]
        iw = idxk.reshape(totcols, 8, 16).transpose(2, 0, 1).reshape(16, totcols * 8)
        idx_wrapped.append(np.tile(iw, (8, 1)))
        segv.append(np.stack([segk.T, vk.T], axis=1))  # [128, 2, totcols]

    return dict(R=R, F=F, col_off=col_off, totcols=totcols,
                idx=idx_wrapped, segv=segv, span_of=span_of)


def pack_weights(cfg, fc_W, fc_b, Wl1, bl1, Wr1, Wl2, bl2, Wr2, head_W, head_b):
    """One [128, WCOLS] f32 tensor. Layout columns:
       0:D        identity [DxD at top-left of 128x128 block]
       D:D+SEGW   iota (0..SEGW-1 replicated per partition)
       then H-wide blocks: fcWT[D,H], Wl1T, Wr1T, Wl2T, Wr2T (H rows),
       headWT col, bl1 col, fcb col, fcb_rep [128? H free]"""
    D, H, SEGW = cfg.D, cfg.H, cfg.SEGW
    cols = {}
    c = 0
    wt = np.zeros((128, D + SEGW + H * 5 + 4 + H), dtype=np.float32)
    wt[:D, 0:D] = np.eye(D, dtype=np.float32)
    cols["ident"] = 0
    c = D
    wt[:, c:c + SEGW] = np.tile(np.arange(SEGW, dtype=np.float32), (128, 1))
    cols["iota"] = c
    c += SEGW
    for name, m in [("fcWT", fc_W.T), ("Wl1T", Wl1.T), ("Wr1T", Wr1.T),
                    ("Wl2T", Wl2.T), ("Wr2T", Wr2.T)]:
        r, cc = m.shape
        wt[:r, c:c + cc] = m
        cols[name] = c
        c += cc
    wt[:H, c] = head_W[0]
    cols["headWT"] = c
    c += 1
    wt[:H, c] = bl1
    cols["bl1"] = c
    c += 1
    wt[:H, c] = fc_b
    cols["fcb"] = c
    c += 1
    wt[:, c:c + H] = np.tile(fc_b, (128, 1))
    cols["fcb_rep"] = c
    c += H
    head_b_eff = float(head_b[0] + head_W[0] @ bl2)
    wt[0, c] = head_b_eff
    cols["hbe"] = c
    c += 1
    return wt, cols, head_b_eff


def build_phase(cfg, pl, wcols, wcols_n, phase, head_b_eff=0.0, reps=1):
    """phase 'A': inputs x, x_my, idx, segv, wt -> output h1_my [NPC, H]
       phase 'B': inputs h1_full, h1T_my, idx, segv, wt -> output out [1, NPC]"""
    N, D, H, NPC, WIN, NWIN = cfg.N, cfg.D, cfg.H, cfg.NPC, cfg.WIN, cfg.NWIN
    NCHUNK, CHUNK, SEGW, COLB, XB = cfg.NCHUNK, cfg.CHUNK, cfg.SEGW, cfg.COLB, cfg.XB
    R, F, col_off, totcols = pl["R"], pl["F"], pl["col_off"], pl["totcols"]
    span_of = pl["span_of"]

    nc = bacc.Bacc(None, target_bir_lowering=False, num_swdge_queues=NSWQ)
    wt_d = nc.dram_tensor("wt", [128, wcols_n], F32, kind="ExternalInput")
    idx_d = nc.dram_tensor("idx", [128, totcols * 8], I16, kind="ExternalInput")
    segv_d = nc.dram_tensor("segv", [128, 2, totcols], F32, kind="ExternalInput")

    if phase == "A":
        x_d = nc.dram_tensor("x", [N, D], F32, kind="ExternalInput")
        xmy_d = nc.dram_tensor("x_my", [NPC, D], F32, kind="ExternalInput")
        h1_d = nc.dram_tensor("h1_my", [NPC, H], F32, kind="ExternalOutput")
        htab_d = nc.dram_tensor("htab", [N, H], F32, kind="Internal")
    else:
        htab_d = nc.dram_tensor("h1_full", [N, H], F32, kind="ExternalInput")
        hTmy_d = nc.dram_tensor("h1T_my", [H, NPC], F32, kind="ExternalInput")
        out_d = nc.dram_tensor("out", [1, NPC], F32, kind="ExternalOutput")

    with TileContext(nc) as tc:
        with (
            tc.tile_pool(name="const", bufs=1) as constp,
            tc.tile_pool(name="resid", bufs=1) as resid,
            tc.tile_pool(name="io", bufs=3) as iop,
            tc.tile_pool(name="gat", bufs=2) as gatp,
            tc.tile_pool(name="bb", bufs=3) as bbp,
            tc.tile_pool(name="ps", bufs=4, space="PSUM") as psp,
            tc.tile_pool(name="ps2", bufs=2, space="PSUM") as psp2,
        ):
            wt = constp.tile([128, wcols_n], F32)
            nc.sync.dma_start(wt[:], wt_d[:, :])
            ident = wt[:, wcols["ident"]:wcols["ident"] + D]
            iota = wt[:, wcols["iota"]:wcols["iota"] + SEGW]
            fcWT = wt[:D, wcols["fcWT"]:wcols["fcWT"] + H]
            Wl1T = wt[:H, wcols["Wl1T"]:wcols["Wl1T"] + H]
            Wr1T = wt[:H, wcols["Wr1T"]:wcols["Wr1T"] + H]
            Wl2T = wt[:H, wcols["Wl2T"]:wcols["Wl2T"] + H]
            Wr2T = wt[:H, wcols["Wr2T"]:wcols["Wr2T"] + H]
            headWT = wt[:H, wcols["headWT"]:wcols["headWT"] + 1]
            bl1 = wt[:H, wcols["bl1"]:wcols["bl1"] + 1]
            fcb = wt[:H, wcols["fcb"]:wcols["fcb"] + 1]
            fcb_rep = wt[:, wcols["fcb_rep"]:wcols["fcb_rep"] + H]

            hT = resid.tile([H, NPC], F32)  # transposed features of my nodes

            for _rep in range(reps):
                if phase == "A":
                    # ---- fc stage: full h table (replicated across cores) ----
                    nt = math.ceil(N / 128)
                    for i0 in range(0, nt, XB):
                        nb = min(XB, nt - i0)
                        rows0 = i0 * 128
                        rowsn = min(N - rows0, nb * 128)
                        xb_t = iop.tile([128, XB, D], F32, tag="xb")
                        nfull = rowsn // 128
                        if nfull:
                            nc.sync.dma_start(
                                xb_t[:, :nfull, :],
                                x_d[rows0:rows0 + nfull * 128, :].rearrange(
                                    "(t p) d -> p t d", p=128))
                        rem = rowsn - nfull * 128
                        if rem:
                            nc.sync.dma_start(xb_t[:rem, nfull, :],
                                              x_d[rows0 + nfull * 128:rows0 + rowsn, :])
                        hb_t = iop.tile([128, XB, H], F32, tag="hb")
                        for t in range(nb):
                            rows = min(128, N - rows0 - t * 128)
                            xT_ps = psp.tile([128, D], F32, space="PSUM", tag="s")
                            nc.tensor.transpose(xT_ps[:, :rows], xb_t[:rows, t, :],
                                                ident[:rows, :rows])
                            xT = iop.tile([128, D], F32, tag="xT_sb")
                            nc.vector.tensor_copy(xT[:], xT_ps[:])
                            h_ps = psp.tile([128, H], F32, space="PSUM", tag="s")
                            nc.tensor.matmul(out=h_ps[:rows, :], lhsT=xT[:, :rows],
                                             rhs=fcWT, start=True, stop=True)
                            nc.vector.tensor_tensor(out=hb_t[:rows, t, :], in0=h_ps[:rows, :],
                                                    in1=fcb_rep[:rows, :],
                                                    op=mybir.AluOpType.add)
                            nc.scalar.activation(hb_t[:rows, t, :], hb_t[:rows, t, :],
                                                 mybir.ActivationFunctionType.Relu)
                        if nfull:
                            nc.sync.dma_start(
                                htab_d[rows0:rows0 + nfull * 128, :].rearrange(
                                    "(t p) d -> p t d", p=128),
                                hb_t[:, :nfull, :])
                        if rem:
                            nc.sync.dma_start(htab_d[rows0 + nfull * 128:rows0 + rowsn, :],
                                              hb_t[:rem, nfull, :])

                    # ---- hT for my nodes (recompute from x_my) ----
                    for i in range(math.ceil(NPC / 128)):
                        rows = min(128, NPC - i * 128)
                        xm_t = iop.tile([128, D], F32, tag="xm")
                        nc.sync.dma_start(xm_t[:rows, :], xmy_d[i * 128:i * 128 + rows, :])
                        xT_ps = psp.tile([128, D], F32, space="PSUM", tag="s")
                        nc.tensor.transpose(xT_ps[:, :rows], xm_t[:rows, :], ident[:rows, :rows])
                        xT = iop.tile([128, D], F32, tag="xT_sb")
                        nc.vector.tensor_copy(xT[:], xT_ps[:])
                        hT_ps = psp.tile([H, 128], F32, space="PSUM", tag="s")
                        nc.tensor.matmul(out=hT_ps[:, :rows], lhsT=fcWT, rhs=xT[:, :rows],
                                         start=True, stop=True)
                        nc.scalar.activation(hT[:, i * 128:i * 128 + rows], hT_ps[:, :rows],
                                             mybir.ActivationFunctionType.Relu, bias=fcb)
                else:
                    nc.sync.dma_start(hT[:], hTmy_d[:, :])
                    out_sb = resid.tile([1, NPC], F32)

                WlT = Wl1T if phase == "A" else Wl2T
                WrT = Wr1T if phase == "A" else Wr2T

                # ---- window loop: gather + segment-sum + SAGE linear ----
                for w in range(NWIN):
                    span = span_of(w)
                    cw0 = col_off[w][0]
                    cwn = col_off[w][NCHUNK - 1] + R[w][NCHUNK - 1] - cw0

                    idx_w = iop.tile([128, cwn * 8], I16, tag="idxw")
                    nc.sync.dma_start(idx_w[:], idx_d[:, cw0 * 8:(cw0 + cwn) * 8])
                    segv_w = iop.tile([128, 2, cwn], F32, tag="segvw")
                    nc.sync.dma_start(segv_w[:], segv_d[:, :, cw0:cw0 + cwn])

                    agg_ps = psp2.tile([H, WIN], F32, space="PSUM", tag="agg")
                    nc.vector.memset(agg_ps[:, :], 0.0)

                    mm = []
                    for c in range(NCHUNK):
                        r = R[w][c]
                        o = col_off[w][c] - cw0  # local col offset within window
                        wg = gatp.tile([128, r, H], F32, tag=f"wg{c % 2}")
                        nc.gpsimd.dma_gather(
                            out_ap=wg[:, :, :],
                            in_ap=htab_d[c * CHUNK:min((c + 1) * CHUNK, N), :],
                            idxs_ap=idx_w[:, o * 8:(o + r) * 8],
                            num_idxs=r * 128, num_idxs_reg=r * 128, elem_size=H,
                            single_packet=False,
                            queue_num=(w * NCHUNK + c) % NSWQ)
                        for b0 in range(0, r, COLB):
                            nb = min(COLB, r - b0)
                            B = bbp.tile([128, COLB, SEGW], F32, tag="B")
                            seg_b = segv_w[:, 0, o + b0:o + b0 + nb].unsqueeze(2) \
                                .to_broadcast([128, nb, SEGW])
                            v_b = segv_w[:, 1, o + b0:o + b0 + nb].unsqueeze(2) \
                                .to_broadcast([128, nb, SEGW])
                            iota_b = iota.unsqueeze(1).to_broadcast([128, nb, SEGW])
                            nc.vector.tensor_tensor(out=B[:, :nb, :], in0=seg_b, in1=iota_b,
                                                    op=mybir.AluOpType.is_equal)
                            nc.vector.tensor_tensor(out=B[:, :nb, :], in0=B[:, :nb, :],
                                                    in1=v_b, op=mybir.AluOpType.mult)
                            for t in range(nb):
                                j = b0 + t
                                f = F[w][c][j]
                                sw = min(SEGW, span - f)
                                mm.append((wg, j, B, t, f, sw))
                    for q, (wg, j, B, t, f, sw) in enumerate(mm):
                        nc.tensor.matmul(out=agg_ps[:, f:f + sw], lhsT=wg[:, j, :],
                                         rhs=B[:, t, :sw], start=False,
                                         stop=(q == len(mm) - 1), skip_group_check=True)

                    aggT = iop.tile([H, WIN], F32, tag="aggT")
                    nc.scalar.copy(aggT[:, :span], agg_ps[:, :span])

                    z_ps = psp2.tile([H, WIN], F32, space="PSUM", tag="z")
                    nc.tensor.matmul(out=z_ps[:, :span], lhsT=WlT, rhs=aggT[:, :span],
                                     start=True, stop=False)
                    nc.tensor.matmul(out=z_ps[:, :span], lhsT=WrT,
                                     rhs=hT[:, w * WIN:w * WIN + span],
                                     start=False, stop=True)

                    if phase == "A":
                        h1T_sb = iop.tile([H, WIN], F32, tag="h1T")
                        nc.scalar.activation(h1T_sb[:, :span], z_ps[:, :span],
                                             mybir.ActivationFunctionType.Relu, bias=bl1)
                        for b in range(math.ceil(span / 128)):
                            rows = min(128, span - b * 128)
                            tr_ps = psp.tile([128, H], F32, space="PSUM", tag="s")
                            nc.tensor.transpose(tr_ps[:rows, :], h1T_sb[:, b * 128:b * 128 + rows],
                                                ident[:H, :H])
                            h1_t = iop.tile([128, H], F32, tag="h1t")
                            nc.vector.tensor_copy(h1_t[:rows, :], tr_ps[:rows, :])
                            nc.sync.dma_start(
                                h1_d[w * WIN + b * 128:w * WIN + b * 128 + rows, :],
                                h1_t[:rows, :])
                    else:
                        h2T_sb = iop.tile([H, WIN], F32, tag="h2T")
                        nc.scalar.copy(h2T_sb[:, :span], z_ps[:, :span])
                        o_ps = psp.tile([1, WIN], F32, space="PSUM", tag="s")
                        nc.tensor.matmul(out=o_ps[:, :span], lhsT=headWT,
                                         rhs=h2T_sb[:, :span], start=True, stop=True)
                        nc.scalar.activation(out_sb[:, w * WIN:w * WIN + span],
                                             o_ps[:, :span],
                                             mybir.ActivationFunctionType.Identity,
                                             bias=wt[0:1, wcols["hbe"]:wcols["hbe"] + 1])

            if phase == "B":
                nc.sync.dma_start(out_d[:, :], out_sb[:, :])

    nc.compile()
    return nc


def kernel(x, edge_index, batch, fc_W, fc_b, Wl1, bl1, Wr1, Wl2, bl2, Wr2,
           head_W, head_b, cfg=None):
    cfg = cfg or Cfg(n_nodes=x.shape[0], in_dim=x.shape[1], hid=fc_W.shape[0])
    x = np.asarray(x, dtype=np.float32)
    src = np.asarray(edge_index[0], dtype=np.int64)
    dst = np.asarray(edge_index[1], dtype=np.int64)
    fc_W, fc_b, Wl1, bl1, Wr1, Wl2, bl2, Wr2, head_W, head_b = [
        np.asarray(a, dtype=np.float32)
        for a in (fc_W, fc_b, Wl1, bl1, Wr1, Wl2, bl2, Wr2, head_W, head_b)]

    t0 = time.time()
    pl = plan(cfg, src, dst)
    timings["plan_s"] = time.time() - t0
    wt, wcols, head_b_eff = pack_weights(cfg, fc_W, fc_b, Wl1, bl1, Wr1,
                                         Wl2, bl2, Wr2, head_W, head_b)

    t0 = time.time()
    ncA = build_phase(cfg, pl, wcols, wt.shape[1], "A")
    timings["buildA_s"] = time.time() - t0
    in_maps_A = []
    for k in range(cfg.NC):
        in_maps_A.append({
            "wt": wt, "idx": pl["idx"][k], "segv": pl["segv"][k],
            "x": x, "x_my": x[k * cfg.NPC:(k + 1) * cfg.NPC],
        })
    t0 = time.time()
    resA = run_bass_kernel_spmd(ncA, in_maps_A, core_ids=list(range(cfg.NC)))
    timings["runA_s"] = time.time() - t0
    timings["hwA_ns"] = resA.exec_time_ns
    h1 = np.concatenate([r["h1_my"] for r in resA.results], axis=0)

    t0 = time.time()
    ncB = build_phase(cfg, pl, wcols, wt.shape[1], "B", head_b_eff=head_b_eff)
    timings["buildB_s"] = time.time() - t0
    in_maps_B = []
    for k in range(cfg.NC):
        in_maps_B.append({
            "wt": wt, "idx": pl["idx"][k], "segv": pl["segv"][k],
            "h1_full": h1,
            "h1T_my": np.ascontiguousarray(h1[k * cfg.NPC:(k + 1) * cfg.NPC].T),
        })
    t0 = time.time()
    resB = run_bass_kernel_spmd(ncB, in_maps_B, core_ids=list(range(cfg.NC)))
    timings["runB_s"] = time.time() - t0
    timings["hwB_ns"] = resB.exec_time_ns
    out = np.concatenate([r["out"][0] for r in resB.results], axis=0)
    return out.reshape(cfg.N, 1).astype(np.float32)

